# revision 3
# baseline (speedup 1.0000x reference)
"""ExpanderLinear on 8 TRN2 NeuronCores.

y = x @ (weight * mask)^T + bias
  x      [8192, 4096] f32
  weight [4096, 4096] f32
  mask   [4096, 4096] i32 (0/1)
  bias   [4096]       f32
  y      [8192, 4096] f32

Sharding: 2D 4x2 grid — 4 token shards x 2 outdim shards. Core k handles
tokens [2048*(k//2), +2048) and outdim [2048*(k%2), +2048). Each core reads
x-shard + w/mask-shard (f32/i32), computes everything on device:
  1. x prep: DRAM->DRAM SWDGE cast DMA f32->bf16 (x_bf, [512,4096] panels)
  2. wm prep: load w (f32) + mask (i32->f32 cast DMA), DVE multiply -> bf16,
     store wm_bf to DRAM
  3. main: DMA-transpose (xbar) loads deliver x^T and wm^T tiles with the
     contraction dim (indim) on partitions; bf16 matmuls accumulate f32 in
     PSUM over 32 k-tiles; DVE adds bias during PSUM->SBUF eviction.
x^T stays resident in SBUF (16MB) across the 4 outdim sweeps.
"""
import os
import sys

sys.path.insert(0, "/opt/trn_rl_repo")

import numpy as np  # noqa: E402

import concourse.bass as bass  # noqa: E402,F401
import concourse.mybir as mybir  # noqa: E402
import concourse.tile as tile  # noqa: E402
import concourse.bacc as bacc  # noqa: E402
from concourse.bass_utils import run_bass_kernel_spmd  # noqa: E402
from concourse.bass_interp import get_hw_module  # noqa: E402

TOKENS, INDIM, OUTDIM = 8192, 4096, 4096
R_SHARDS, C_SHARDS = 4, 2
T_C, O_C = TOKENS // R_SHARDS, OUTDIM // C_SHARDS  # 2048, 2048

P = 128     # partitions / k-tile size
OG = 512    # outdim per sweep (= psum free dim)
TPAN = 512  # token panel (DMAT granularity)
KC = 1024   # prep chunk width


def build_program(t_c=T_C, o_c=O_C, k=INDIM):
    KT = k // P
    n_og = o_c // OG
    n_tp = t_c // TPAN
    n_ts = TPAN // P
    n_kc = k // KC

    nc = bacc.Bacc("TRN2", target_bir_lowering=False, debug=False,
                   num_devices=8)
    x = nc.dram_tensor("x", [t_c, k], mybir.dt.float32, kind="ExternalInput")
    w = nc.dram_tensor("w", [o_c, k], mybir.dt.float32, kind="ExternalInput")
    m = nc.dram_tensor("m", [o_c, k], mybir.dt.int32, kind="ExternalInput")
    b = nc.dram_tensor("b", [o_c], mybir.dt.float32, kind="ExternalInput")
    y = nc.dram_tensor("y", [t_c, o_c], mybir.dt.float32,
                       kind="ExternalOutput")

    with tile.TileContext(nc) as tc:
        with (tc.tile_pool(name="dram", bufs=1, space="DRAM") as dram,
              tc.tile_pool(name="xT_pool", bufs=1) as xT_pool,
              tc.tile_pool(name="wmT_pool", bufs=1) as wmT_pool,
              tc.tile_pool(name="prep", bufs=2) as prep,
              tc.tile_pool(name="wmc", bufs=2) as wmc,
              tc.tile_pool(name="outp", bufs=3) as outp,
              tc.tile_pool(name="biasp", bufs=2) as biasp,
              tc.tile_pool(name="psum", bufs=4, space="PSUM") as psum_pool):
            x_bf = [dram.tile([TPAN, k], mybir.dt.bfloat16,
                              name=f"x_bf{i}", tag=f"x_bf{i}")
                    for i in range(n_tp)]
            wm_bf = [dram.tile([OG, k], mybir.dt.bfloat16,
                               name=f"wm_bf{i}", tag=f"wm_bf{i}")
                     for i in range(n_og)]
            xT = xT_pool.tile([P, KT, t_c], mybir.dt.bfloat16, name="xT")

            # x prep: DRAM->DRAM cast f32 -> bf16 (SWDGE)
            for tp in range(n_tp):
                nc.gpsimd.dma_start(x_bf[tp][:, :],
                                    x[tp * TPAN:(tp + 1) * TPAN, :])

            for og in range(n_og):
                # wm prep for this outdim sweep
                for op_ in range(OG // P):
                    ro = og * OG + op_ * P
                    for kc in range(n_kc):
                        ks = slice(kc * KC, (kc + 1) * KC)
                        wch = prep.tile([P, KC], mybir.dt.float32,
                                        tag="wchunk")
                        nc.sync.dma_start(wch[:, :], w[ro:ro + P, ks])
                        mch = prep.tile([P, KC], mybir.dt.float32,
                                        tag="mchunk")
                        nc.gpsimd.dma_start(mch[:, :], m[ro:ro + P, ks])
                        wmch = wmc.tile([P, KC], mybir.dt.bfloat16,
                                        tag="wmchunk")
                        nc.vector.tensor_mul(wmch[:, :], wch[:, :], mch[:, :])
                        nc.sync.dma_start(
                            wm_bf[og][op_ * P:(op_ + 1) * P, ks], wmch[:, :])

                # wm^T delivery via xbar DMA transpose
                wmT = wmT_pool.tile([P, KT, OG], mybir.dt.bfloat16, tag="wmT")
                for kt in range(KT):
                    nc.sync.dma_start(wmT[:, kt, :],
                                      wm_bf[og][:, kt * P:(kt + 1) * P],
                                      transpose=True)

                bias_t = biasp.tile([P, OG], mybir.dt.float32, tag="bias")
                nc.sync.dma_start(
                    bias_t[:, :],
                    b[None, og * OG:(og + 1) * OG].to_broadcast((P, OG)))

                # x^T delivery (first sweep only; tile stays resident)
                if og == 0:
                    for tp in range(n_tp):
                        for kt in range(KT):
                            nc.sync.dma_start(
                                xT[:, kt, tp * TPAN:(tp + 1) * TPAN],
                                x_bf[tp][:, kt * P:(kt + 1) * P],
                                transpose=True)

                for tp in range(n_tp):
                    for ts_ in range(n_ts):
                        t0 = tp * TPAN + ts_ * P
                        pt = psum_pool.tile([P, OG], mybir.dt.float32,
                                            tag="acc")
                        for kt in range(KT):
                            nc.tensor.matmul(pt[:, :], xT[:, kt, t0:t0 + P],
                                             wmT[:, kt, :],
                                             start=(kt == 0),
                                             stop=(kt == KT - 1))
                        ot = outp.tile([P, OG], mybir.dt.float32, tag="out")
                        nc.vector.tensor_tensor(
                            out=ot[:, :], in0=pt[:, :], in1=bias_t[:, :],
                            op=mybir.AluOpType.add)
                        nc.sync.dma_start(
                            y[t0:t0 + P, og * OG:(og + 1) * OG], ot[:, :])

    nc.compile()
    nc.m = get_hw_module(nc.m)
    return nc


_PROGRAM = None


def _get_program():
    global _PROGRAM
    if _PROGRAM is None:
        _PROGRAM = build_program()
    return _PROGRAM


def _enable_tracing():
    """Install the axon NTFF profile hook if the image's antenv lacks it.

    Returns True when tracing should work. Replicates
    trn_agent_boot.trn_boot's ctypes hook against libaxon_pjrt.so, and
    stubs out the S3 artifact upload (not available here).
    """
    try:
        import contextlib
        import ctypes
        import types

        import concourse.bass_utils as bu
        bu.upload_artifacts = lambda tmpdir: ""  # no S3 in this container

        try:
            from antenv.axon_hooks import get_axon_ntff_profile_hook
            if get_axon_ntff_profile_hook() is not None:
                return True
        except ImportError:
            pass

        so_path = "/opt/axon/libaxon_pjrt.so"
        if not os.path.exists(so_path):
            return False
        lib = ctypes.CDLL(so_path)
        if not hasattr(lib, "axon_start_nrt_profile"):
            return False
        lib.axon_start_nrt_profile.argtypes = [
            ctypes.POINTER(ctypes.c_int64), ctypes.c_size_t]
        lib.axon_start_nrt_profile.restype = ctypes.c_int64
        lib.axon_stop_nrt_profile.argtypes = [ctypes.c_char_p]
        lib.axon_stop_nrt_profile.restype = ctypes.c_int64

        @contextlib.contextmanager
        def _hook(output_dir, device_ids):
            import jax
            jax.devices()
            if device_ids:
                ids = (ctypes.c_int64 * len(device_ids))(*device_ids)
                rc = lib.axon_start_nrt_profile(ids, len(device_ids))
            else:
                rc = lib.axon_start_nrt_profile(None, 0)
            if rc != 0:
                raise RuntimeError(f"axon_start_nrt_profile rc={rc}")
            try:
                yield
            finally:
                n = lib.axon_stop_nrt_profile(str(output_dir).encode())
                if n <= 0:
                    print(f"ntff profile: rc={n} (no files) -> {output_dir}")

        mod = types.ModuleType("antenv.axon_hooks")
        _state = {"hook": _hook}
        mod.set_axon_ntff_profile_hook = lambda h: _state.update(hook=h)
        mod.get_axon_ntff_profile_hook = lambda: _state["hook"]
        import antenv
        sys.modules["antenv.axon_hooks"] = mod
        antenv.axon_hooks = mod
        return True
    except Exception as e:  # tracing is best-effort
        print(f"tracing unavailable: {e}")
        return False


def kernel(x, weight, bias, mask):
    x = np.asarray(x, dtype=np.float32)
    weight = np.asarray(weight, dtype=np.float32)
    bias = np.asarray(bias, dtype=np.float32)
    mask = np.asarray(mask, dtype=np.int32)

    nc = _get_program()

    in_maps = []
    for core in range(8):
        r, c = core // C_SHARDS, core % C_SHARDS
        in_maps.append({
            "x": np.ascontiguousarray(x[r * T_C:(r + 1) * T_C]),
            "w": np.ascontiguousarray(weight[c * O_C:(c + 1) * O_C]),
            "m": np.ascontiguousarray(mask[c * O_C:(c + 1) * O_C]),
            "b": np.ascontiguousarray(bias[c * O_C:(c + 1) * O_C]),
        })

    trace = os.environ.get("KERNEL_TRACE", "1") == "1"
    if trace:
        trace = _enable_tracing()
    res = None
    if trace:
        tmpdir = os.environ.get("KERNEL_TRACE_DIR")
        if tmpdir:
            os.makedirs(tmpdir, exist_ok=True)
        try:
            res = run_bass_kernel_spmd(nc, in_maps, core_ids=list(range(8)),
                                       trace=True, tmpdir=tmpdir)
        except Exception as e:
            print(f"traced run failed ({e!r}); rerunning untraced")
            res = None
    if res is None:
        res = run_bass_kernel_spmd(nc, in_maps, core_ids=list(range(8)))
    if res.exec_time_ns is not None:
        print(f"HW exec time: {res.exec_time_ns} ns")

    out = np.empty((TOKENS, OUTDIM), dtype=np.float32)
    for core in range(8):
        r, c = core // C_SHARDS, core % C_SHARDS
        out[r * T_C:(r + 1) * T_C, c * O_C:(c + 1) * O_C] = \
            res.results[core]["y"]
    return out
